# revision 5
# baseline (speedup 1.0000x reference)
"""CNOT permutation kernel for Trainium2 (Bass), 8-core data parallel.

Problem (hardcoded from spec): state (16, 2**24) f32, control=3, target=10,
num_qubits=24.  With c2 = 24-3-1 = 20 and t2 = 24-10-1 = 13:

    out[b, j] = state[b, j ^ (1<<13)]  if (j >> 20) & 1 else state[b, j]

Pure data movement.  Viewing the per-core shard flat (row stride 2**24 is a
multiple of the 2**21 control-bit period, so both rows fuse) as
[blk:16, ctrl:2, c:64, d:2, e:8192]:

    out[blk, 0, c, d, e] = in[blk, 0, c, d, e]      (identity half)
    out[blk, 1, c, d, e] = in[blk, 1, c, 1-d, e]    (swap 8192-elem chunk pairs)

Sharding: batch axis, 2 rows per core (pure data parallel).

Device kernels are HBM->SBUF->HBM bounces (direct DRAM->DRAM DMA serializes
read+write inside each SDMA engine; split load/store descriptors stream at
the SBUF-AXI fabric rate ~435 GB/s/core).  Loads issue on the Sync HWDGE
ring, stores on the Scalar ring; the 16 SDMA engines round-robin between
rings at packet granularity so both HBM directions stay busy.

Pipeline structure: units are one load + one store each; the chunk-pair
swap is folded into the load AP (middle dim with stride -TBIT), so HBM
writes are fully sequential.  SBUF ring of RING 32 KiB/partition slots,
two semaphores (RAW: store waits its unit's load; WAR: load waits the
store that previously used its slot).

Tail shape (profiled): the last ring store can only start after the last
load's completion receipt (~2 us), and the trace showed ~10 us of
store-only drain at reduced engine concurrency.  Fix: the FINAL region is
PRESTAGED -- its load is issued first (into a dedicated SBUF slot) and its
store is issued last with a trivially-satisfied wait, so the tail is a
fully-queued 4 MiB store draining at the full 16-engine rate while the
last ring store's receipt latency hides behind it.  Steady state is at the
hard per-port ceiling (~26.9 GB/s x 16 engines, 610 ns per 16 KiB packet),
so edges are all that's left to optimize.

1. In-place (default): a single DRAM tensor, pre-initialized with the input
   shard by donating it as the PJRT output buffer (the same donation
   mechanism run_bass_via_pjrt uses for its zero-filled outputs).  Only the
   control-bit=1 half is touched: 64 MiB read + 64 MiB write per core.  The
   identity half is never moved at all.
2. Full-copy (fallback): separate in/out tensors, 128+128 MiB per core.

kernel() runs the in-place path and sample-checks the permutation against
the host input; if the donation aliasing ever fails to hold (output buffer
not seeded with the input), it falls back to the full-copy path.
"""

import numpy as np

import concourse.bass as bass
import concourse.mybir as mybir
from concourse.bass_utils import run_bass_kernel_spmd

NUM_QUBITS = 24
DIM = 1 << NUM_QUBITS
BATCH = 16
N_CORES = 8
ROWS = BATCH // N_CORES  # 2 rows per core
C2 = NUM_QUBITS - 3 - 1  # 20
T2 = NUM_QUBITS - 10 - 1  # 13
CBIT = 1 << C2  # 1048576 elements (4 MiB)
TBIT = 1 << T2  # 8192 elements (32 KiB)
BLK = 2 * CBIT  # control-bit period
NBLK = ROWS * DIM // BLK  # 16 blocks in the fused per-core space

P = 128
PAIRS = CBIT // (2 * TBIT)  # 64 chunk pairs per 4 MiB region

# Units are e-axis slices (each spans all 128 partitions).  BODY_W elems /
# partition per unit in steady state; the first region ramps up so the
# store stream starts early.
BODY_W = 8192  # 4 MiB units: body stores collapse to one contiguous run
NSLOT = 6  # SBUF slots: 6 x 32 KiB/partition = 192 KiB of ~208 usable
RING = NSLOT - 1  # 5-deep ring; slot RING is the prestage slot
FREE = BODY_W
# Head ramp: w=4096 keeps descriptors at 16 KiB (w<2048 makes 2-8 KiB
# descriptors that crawl under neighbor-core contention).
HEAD_SPLIT = [4096, 4096]
TAIL_SPLIT = [4096, 4096]  # fallback (full-copy) path only

_cache = {}


def _region_units(kind, base, splits):
    out, e0 = [], 0
    for w in splits:
        out.append((kind, base, e0, w))
        e0 += w
    assert e0 == TBIT
    return out


def _units(inplace):
    """Yield (kind, region_base, e0, w): the unit covers the e in [e0,e0+w)
    slice of all 128 TBIT-chunks of its 4 MiB region.  kind 's'=swap,
    'i'=identity."""
    regions = []
    for b in range(NBLK):
        if not inplace:
            regions.append(("i", b * BLK))
        regions.append(("s", b * BLK + CBIT))
    if not inplace:
        # Ramp regions get strided stores when split mid-chunk -- except
        # identity regions, whose stores stay contiguous at any width.  Put
        # an identity region last so both ramps are identity-only and every
        # swap store is one contiguous 4 MiB run.
        regions.append(regions.pop(-2))
    body = [BODY_W] * (TBIT // BODY_W)
    units = []
    for r, (kind, base) in enumerate(regions):
        if r == 0:
            splits = HEAD_SPLIT
        elif r == len(regions) - 1:
            splits = TAIL_SPLIT
        else:
            splits = body
        units.extend(_region_units(kind, base, splits))
    return units


def _swap_in_ap(src, base, e0, w):
    # partition p = (c, j) reads chunk (c, 1-j)'s e-slice
    return bass.AP(src, base + TBIT + e0, [[2 * TBIT, PAIRS], [-TBIT, 2], [1, w]])


def _swap_out_ap(dst, base, e0, w):
    return bass.AP(dst, base + e0, [[TBIT, P], [1, w]])


def _emit_prestage(nc, src, dst):
    """In-place swap pipeline with the final region prestaged.

    Ring units (region 0 split per HEAD_SPLIT, regions 1..NBLK-2 whole)
    cycle through RING SBUF slots; the final region is loaded FIRST into
    slot RING and stored LAST, so the tail is a fully-queued 4 MiB store
    with no exposed load-completion receipt."""
    ring = []  # (base, e0, w)
    base0 = 0 * BLK + CBIT
    e0 = 0
    for w in HEAD_SPLIT:
        ring.append((base0, e0, w))
        e0 += w
    assert e0 == TBIT
    for b in range(1, NBLK - 1):
        ring.append((b * BLK + CBIT, 0, BODY_W))
    pre = ((NBLK - 1) * BLK + CBIT, 0, BODY_W)  # prestaged final region
    n_units = len(ring) + 1

    with (
        nc.sbuf_tensor("tiles", [P, NSLOT * FREE], mybir.dt.float32) as tiles,
        nc.semaphore("load_sem") as load_sem,
        nc.semaphore("store_sem") as store_sem,
        nc.Block() as block,
    ):

        def tile_view(slot, w):
            s = slot * FREE
            return tiles[:, s : s + w]

        @block.sync
        def _(sync):
            # Prestage load: final region into the dedicated slot.
            base, e0, w = pre
            sync.dma_start(
                out=tile_view(RING, w), in_=_swap_in_ap(src, base, e0, w)
            ).then_inc(load_sem, 16)
            for r, (base, e0, w) in enumerate(ring):
                if r >= RING:
                    # WAR: slot r%RING was last used by ring unit r-RING,
                    # whose store is at position r-RING in the store queue.
                    sync.wait_ge(store_sem, 16 * (r - RING + 1))
                sync.dma_start(
                    out=tile_view(r % RING, w), in_=_swap_in_ap(src, base, e0, w)
                ).then_inc(load_sem, 16)

        @block.scalar
        def _(scalar):
            for r, (base, e0, w) in enumerate(ring):
                # RAW: ring unit r's load is at position r+1 in the load queue.
                scalar.wait_ge(load_sem, 16 * (r + 2))
                scalar.dma_start(
                    out=_swap_out_ap(dst, base, e0, w), in_=tile_view(r % RING, w)
                ).then_inc(store_sem, 16)
            base, e0, w = pre
            scalar.wait_ge(load_sem, 16)  # prestage load (queue head)
            scalar.dma_start(
                out=_swap_out_ap(dst, base, e0, w), in_=tile_view(RING, w)
            ).then_inc(store_sem, 16)
            scalar.wait_ge(store_sem, 16 * n_units)


def _emit_bounce(nc, src, dst, units):
    """Loads on sync / stores on scalar, NSLOT-deep pipeline over units."""
    n = len(units)
    with (
        nc.sbuf_tensor("tiles", [P, NSLOT * FREE], mybir.dt.float32) as tiles,
        nc.semaphore("load_sem") as load_sem,
        nc.semaphore("store_sem") as store_sem,
        nc.Block() as block,
    ):

        def tile_view(i, w):
            s = (i % NSLOT) * FREE
            return tiles[:, s : s + w]

        @block.sync
        def _(sync):
            for i, (kind, base, e0, w) in enumerate(units):
                if i >= NSLOT:
                    sync.wait_ge(store_sem, 16 * (i - NSLOT + 1))
                if kind == "s":
                    # partition p = (c, j) reads chunk (c, 1-j)'s e-slice
                    in_ap = bass.AP(
                        src,
                        base + TBIT + e0,
                        [[2 * TBIT, PAIRS], [-TBIT, 2], [1, w]],
                    )
                else:
                    in_ap = bass.AP(src, base + e0 * P, [[1, w * P]])
                sync.dma_start(out=tile_view(i, w), in_=in_ap).then_inc(
                    load_sem, 16
                )

        @block.scalar
        def _(scalar):
            for i, (kind, base, e0, w) in enumerate(units):
                scalar.wait_ge(load_sem, 16 * (i + 1))
                if kind == "s":
                    out_ap = bass.AP(dst, base + e0, [[TBIT, P], [1, w]])
                else:
                    out_ap = bass.AP(dst, base + e0 * P, [[1, w * P]])
                scalar.dma_start(
                    out=out_ap, in_=tile_view(i, w)
                ).then_inc(store_sem, 16)
            scalar.wait_ge(store_sem, 16 * n)


def _build_nc(inplace):
    nc = bass.Bass(target_bir_lowering=False)
    out = nc.dram_tensor("out", (ROWS, DIM), mybir.dt.float32, kind="ExternalOutput")
    if inplace:
        _emit_prestage(nc, out, out)
    else:
        st = nc.dram_tensor(
            "state", (ROWS, DIM), mybir.dt.float32, kind="ExternalInput"
        )
        _emit_bounce(nc, st, out, _units(inplace=False))
    if not nc.is_finalized():
        nc.finalize()
    return nc


def _get_nc(inplace):
    key = ("ip" if inplace else "fc",)
    if key not in _cache:
        _cache[key] = _build_nc(inplace)
    return _cache[key]


def _run_donated(nc, state):
    """Run `nc` via PJRT shard_map over 8 cores, donating the input state as
    the initial content of the (aliased) output buffer — the same donation
    mechanism run_bass_via_pjrt uses for its zero-filled outputs."""
    import jax
    from jax.experimental.shard_map import shard_map
    from jax.sharding import Mesh, PartitionSpec

    from concourse.bass2jax import (
        _bass_exec_p,
        install_neuronx_cc_hook,
        partition_id_tensor,
    )

    install_neuronx_cc_hook()

    out_names, out_avals = [], []
    for alloc in nc.m.functions[0].allocations:
        if (
            isinstance(alloc, mybir.MemoryLocationSet)
            and alloc.kind == "ExternalOutput"
        ):
            out_names.append(alloc.memorylocations[0].name)
            out_avals.append(
                jax.core.ShapedArray(
                    tuple(alloc.tensor_shape), mybir.dt.np(alloc.dtype)
                )
            )
    partition_name = nc.partition_id_tensor.name if nc.partition_id_tensor else None
    in_names = list(out_names)
    if partition_name is not None:
        in_names.append(partition_name)

    if "donated_fn" not in _cache:

        def _body(buf):
            operands = [buf]
            if partition_name is not None:
                operands.append(partition_id_tensor())
            outs = _bass_exec_p.bind(
                *operands,
                out_avals=tuple(out_avals),
                in_names=tuple(in_names),
                out_names=tuple(out_names),
                lowering_input_output_aliases=(),
                sim_require_finite=True,
                sim_require_nnan=True,
                nc=nc,
            )
            return outs[0]

        devices = jax.devices()[:N_CORES]
        mesh = Mesh(np.asarray(devices), ("core",))
        _cache["donated_fn"] = jax.jit(
            shard_map(
                _body,
                mesh=mesh,
                in_specs=(PartitionSpec("core"),),
                out_specs=PartitionSpec("core"),
                check_rep=False,
            ),
            donate_argnums=(0,),
            keep_unused=True,
        )

    out = _cache["donated_fn"](state)
    return np.asarray(out)


def _sample_ok(state, out, rng, k=2048):
    """Spot-check out[b, j] == state[b, j ^ (1<<13) if bit20(j) else j]."""
    b = rng.integers(0, BATCH, size=k)
    j = rng.integers(0, DIM, size=k)
    src = np.where((j >> C2) & 1 == 1, j ^ TBIT, j)
    return np.array_equal(out[b, j], state[b, src])


def kernel(state, control=3, target=10, num_qubits=24, **_):
    state = np.ascontiguousarray(np.asarray(state, dtype=np.float32))
    assert state.shape == (BATCH, DIM), state.shape
    assert int(control) == 3 and int(target) == 10 and int(num_qubits) == 24

    rng = np.random.default_rng(0)
    try:
        out = _run_donated(_get_nc(inplace=True), state)
        if _sample_ok(state, out, rng):
            return out
    except Exception:
        # Retry once with a fresh jit: a transient dispatch failure before
        # any device execution is cheap to retry; a second failure means the
        # aliasing mechanism is broken here -> full-copy.
        _cache.pop("donated_fn", None)
        try:
            out = _run_donated(_get_nc(inplace=True), state)
            if _sample_ok(state, out, rng):
                return out
        except Exception:
            pass

    # Fallback: full-copy kernel through run_bass_kernel_spmd.
    nc = _get_nc(inplace=False)
    in_maps = [{"state": state[c * ROWS : (c + 1) * ROWS]} for c in range(N_CORES)]
    res = run_bass_kernel_spmd(nc, in_maps, core_ids=list(range(N_CORES)))
    return np.concatenate([r["out"] for r in res.results], axis=0)



# revision 9
# speedup vs baseline: 1.3234x; 1.3234x over previous
"""CNOT permutation kernel for Trainium2 (Bass), 8-core data parallel.

Problem (hardcoded from spec): state (16, 2**24) f32, control=3, target=10,
num_qubits=24.  With c2 = 24-3-1 = 20 and t2 = 24-10-1 = 13:

    out[b, j] = state[b, j ^ (1<<13)]  if (j >> 20) & 1 else state[b, j]

Pure data movement.  Viewing the per-core shard flat (row stride 2**24 is a
multiple of the 2**21 control-bit period, so both rows fuse) as
[blk:16, ctrl:2, c:64, d:2, e:8192]:

    out[blk, 0, c, d, e] = in[blk, 0, c, d, e]      (identity half)
    out[blk, 1, c, d, e] = in[blk, 1, c, 1-d, e]    (swap 8192-elem chunk pairs)

Sharding: batch axis, 2 rows per core (pure data parallel).

Device kernels are HBM->SBUF->HBM bounces (direct DRAM->DRAM DMA serializes
read+write inside each SDMA engine; split load/store descriptors stream at
the SBUF-AXI fabric rate ~435 GB/s/core).  Loads issue on the Sync HWDGE
ring, stores on the Scalar ring; the 16 SDMA engines round-robin between
rings at packet granularity so both HBM directions stay busy.

Pipeline structure: units are one load + one store each; the chunk-pair
swap is folded into the load AP (middle dim with stride -TBIT), so HBM
writes are fully sequential.  SBUF ring of RING 32 KiB/partition slots,
two semaphores (RAW: store waits its unit's load; WAR: load waits the
store that previously used its slot).

Tail shape (profiled): the last ring store can only start after the last
load's completion receipt (~2 us), and the trace showed ~10 us of
store-only drain at reduced engine concurrency.  Fix: the FINAL region is
PRESTAGED -- its load is issued first (into a dedicated SBUF slot) and its
store is issued last with a trivially-satisfied wait, so the tail is a
fully-queued 4 MiB store draining at the full 16-engine rate while the
last ring store's receipt latency hides behind it.  Steady state is at the
hard per-port ceiling (~26.9 GB/s x 16 engines, 610 ns per 16 KiB packet),
so edges are all that's left to optimize.

1. In-place (default): a single DRAM tensor, pre-initialized with the input
   shard by donating it as the PJRT output buffer (the same donation
   mechanism run_bass_via_pjrt uses for its zero-filled outputs).  Only the
   control-bit=1 half is touched: 64 MiB read + 64 MiB write per core.  The
   identity half is never moved at all.
2. Full-copy (fallback): separate in/out tensors, 128+128 MiB per core.

kernel() runs the in-place path and sample-checks the permutation against
the host input; if the donation aliasing ever fails to hold (output buffer
not seeded with the input), it falls back to the full-copy path.
"""

import numpy as np

import concourse.bass as bass
import concourse.mybir as mybir
from concourse.bass_utils import run_bass_kernel_spmd

NUM_QUBITS = 24
DIM = 1 << NUM_QUBITS
BATCH = 16
N_CORES = 8
ROWS = BATCH // N_CORES  # 2 rows per core
C2 = NUM_QUBITS - 3 - 1  # 20
T2 = NUM_QUBITS - 10 - 1  # 13
CBIT = 1 << C2  # 1048576 elements (4 MiB)
TBIT = 1 << T2  # 8192 elements (32 KiB)
BLK = 2 * CBIT  # control-bit period
NBLK = ROWS * DIM // BLK  # 16 blocks in the fused per-core space

P = 128
PAIRS = CBIT // (2 * TBIT)  # 64 chunk pairs per 4 MiB region

# Units are e-axis slices (each spans all 128 partitions).  BODY_W elems /
# partition per unit in steady state; the first region ramps up so the
# store stream starts early.
BODY_W = 8192  # 4 MiB units: body stores collapse to one contiguous run
NSLOT = 6  # SBUF slots: 6 x 32 KiB/partition = 192 KiB of ~208 usable
RING = NSLOT - 1  # 5-deep ring; slot RING is the prestage slot
FREE = BODY_W
MIX_RING = 11  # mixed path: 11 x 16 KiB ring + 16 KiB prestage = 192 KiB
# Head ramp: w=4096 keeps descriptors at 16 KiB (w<2048 makes 2-8 KiB
# descriptors that crawl under neighbor-core contention).
HEAD_SPLIT = [4096, 4096]
TAIL_SPLIT = [4096, 4096]  # fallback (full-copy) path only

_cache = {}


def _region_units(kind, base, splits):
    out, e0 = [], 0
    for w in splits:
        out.append((kind, base, e0, w))
        e0 += w
    assert e0 == TBIT
    return out


def _units(inplace):
    """Yield (kind, region_base, e0, w): the unit covers the e in [e0,e0+w)
    slice of all 128 TBIT-chunks of its 4 MiB region.  kind 's'=swap,
    'i'=identity."""
    regions = []
    for b in range(NBLK):
        if not inplace:
            regions.append(("i", b * BLK))
        regions.append(("s", b * BLK + CBIT))
    if not inplace:
        # Ramp regions get strided stores when split mid-chunk -- except
        # identity regions, whose stores stay contiguous at any width.  Put
        # an identity region last so both ramps are identity-only and every
        # swap store is one contiguous 4 MiB run.
        regions.append(regions.pop(-2))
    body = [BODY_W] * (TBIT // BODY_W)
    units = []
    for r, (kind, base) in enumerate(regions):
        if r == 0:
            splits = HEAD_SPLIT
        elif r == len(regions) - 1:
            splits = TAIL_SPLIT
        else:
            splits = body
        units.extend(_region_units(kind, base, splits))
    return units


def _swap_in_ap(src, base, e0, w):
    # partition p = (c, j) reads chunk (c, 1-j)'s e-slice
    return bass.AP(src, base + TBIT + e0, [[2 * TBIT, PAIRS], [-TBIT, 2], [1, w]])


def _swap_out_ap(dst, base, e0, w):
    return bass.AP(dst, base + e0, [[TBIT, P], [1, w]])


# Mixed-path APs: the A-half of a region (chunks at +0 in each 2*TBIT block)
# bounces through SBUF as a [128, HALF_W] tile (each 32 KiB chunk split
# across 2 partitions so all 16 engine ports engage); the B-half moves
# HBM->HBM directly.
HALF_W = TBIT // 2  # 4096


def _half_dram_ap(t, base):
    # partition p = (c, h): run at base + c*2*TBIT + h*HALF_W, 16 KiB each
    return bass.AP(t, base, [[2 * TBIT, PAIRS], [HALF_W, 2], [1, HALF_W]])


def _d2d_ap(t, base):
    # 64 contiguous 32 KiB runs at stride 2*TBIT
    return bass.AP(t, base, [[2 * TBIT, PAIRS], [1, TBIT]])


def _emit_mixed(nc, buf):
    """In-place swap, half bounced / half direct DRAM->DRAM.

    Per 4 MiB region r (64 pairs of 32 KiB chunks A@+0 / B@+TBIT):
      L_r: A-half (2 MiB) -> SBUF ring slot        [sync queue]
      D_r: B-half -> A-locations, direct HBM->HBM  [scalar queue]
      S_r: SBUF (A data) -> B-locations            [scalar queue]
    with L_r -> D_r -> S_r enforced by completion semaphores.  D2D moves
    payload at ~20.8 GB/s/engine (measured) vs 27.2 streaming, but only
    transits the engine once, so total engine-time is ~0.82x of the pure
    bounce.  The final region is prestaged: its load is first in the sync
    queue, its D2D second in the scalar queue, and its store last, so the
    tail is fully-queued stores at the 16-engine rate.
    """
    n_ring = NBLK - 1  # regions 0..14 through the ring
    pre_base = (NBLK - 1) * BLK + CBIT
    ring_bases = [b * BLK + CBIT for b in range(n_ring)]

    with (
        nc.sbuf_tensor("ring", [P, MIX_RING * HALF_W], mybir.dt.float32) as ring,
        nc.sbuf_tensor("pre", [P, HALF_W], mybir.dt.float32) as pre,
        nc.semaphore("load_sem") as load_sem,
        nc.semaphore("d2d_sem") as d2d_sem,
        nc.semaphore("store_sem") as store_sem,
        nc.Block() as block,
    ):

        def slot(r):
            s = (r % MIX_RING) * HALF_W
            return ring[:, s : s + HALF_W]

        @block.sync
        def _(sync):
            # Prestage load: final region's A-half into the dedicated tile.
            sync.dma_start(
                out=pre[:, :], in_=_half_dram_ap(buf, pre_base)
            ).then_inc(load_sem, 16)
            for r, base in enumerate(ring_bases):
                if r >= MIX_RING:
                    # WAR: slot last used by ring unit r-MIX_RING, whose
                    # store is at position r-MIX_RING in store completion.
                    sync.wait_ge(store_sem, 16 * (r - MIX_RING + 1))
                sync.dma_start(
                    out=slot(r), in_=_half_dram_ap(buf, base)
                ).then_inc(load_sem, 16)

        @block.scalar
        def _(scalar):
            # Prestage D2D: B-half -> A-locations of the final region.
            scalar.wait_ge(load_sem, 16)  # prestage load (sync queue head)
            scalar.dma_start(
                out=_d2d_ap(buf, pre_base), in_=_d2d_ap(buf, pre_base + TBIT)
            ).then_inc(d2d_sem, 16)
            # Interleave D_r / S_{r-1} with lag 1 so receipt waits are
            # resolved by the time the sequencer reaches them.
            for r, base in enumerate(ring_bases):
                # D_r needs L_r complete (position r+1 in the load queue).
                scalar.wait_ge(load_sem, 16 * (r + 2))
                scalar.dma_start(
                    out=_d2d_ap(buf, base), in_=_d2d_ap(buf, base + TBIT)
                ).then_inc(d2d_sem, 16)
                if r >= 1:
                    pb = ring_bases[r - 1]
                    # S_{r-1} needs D_{r-1} complete (position r in d2d).
                    scalar.wait_ge(d2d_sem, 16 * (r + 1))
                    scalar.dma_start(
                        out=_half_dram_ap(buf, pb + TBIT), in_=slot(r - 1)
                    ).then_inc(store_sem, 16)
            # Last ring store.
            scalar.wait_ge(d2d_sem, 16 * (n_ring + 1))
            scalar.dma_start(
                out=_half_dram_ap(buf, ring_bases[-1] + TBIT), in_=slot(n_ring - 1)
            ).then_inc(store_sem, 16)
            # Prestage store: A data -> B-locations of the final region.
            scalar.wait_ge(d2d_sem, 16)  # prestage D2D (scalar queue head)
            scalar.dma_start(
                out=_half_dram_ap(buf, pre_base + TBIT), in_=pre[:, :]
            ).then_inc(store_sem, 16)
            scalar.wait_ge(store_sem, 16 * NBLK)


def _emit_prestage(nc, src, dst):
    """In-place swap pipeline with the final region prestaged.

    Ring units (region 0 split per HEAD_SPLIT, regions 1..NBLK-2 whole)
    cycle through RING SBUF slots; the final region is loaded FIRST into
    slot RING and stored LAST, so the tail is a fully-queued 4 MiB store
    with no exposed load-completion receipt."""
    ring = []  # (base, e0, w)
    base0 = 0 * BLK + CBIT
    e0 = 0
    for w in HEAD_SPLIT:
        ring.append((base0, e0, w))
        e0 += w
    assert e0 == TBIT
    for b in range(1, NBLK - 1):
        ring.append((b * BLK + CBIT, 0, BODY_W))
    pre = ((NBLK - 1) * BLK + CBIT, 0, BODY_W)  # prestaged final region
    n_units = len(ring) + 1

    with (
        nc.sbuf_tensor("tiles", [P, NSLOT * FREE], mybir.dt.float32) as tiles,
        nc.semaphore("load_sem") as load_sem,
        nc.semaphore("store_sem") as store_sem,
        nc.Block() as block,
    ):

        def tile_view(slot, w):
            s = slot * FREE
            return tiles[:, s : s + w]

        @block.sync
        def _(sync):
            # Prestage load: final region into the dedicated slot.
            base, e0, w = pre
            sync.dma_start(
                out=tile_view(RING, w), in_=_swap_in_ap(src, base, e0, w)
            ).then_inc(load_sem, 16)
            for r, (base, e0, w) in enumerate(ring):
                if r >= RING:
                    # WAR: slot r%RING was last used by ring unit r-RING,
                    # whose store is at position r-RING in the store queue.
                    sync.wait_ge(store_sem, 16 * (r - RING + 1))
                sync.dma_start(
                    out=tile_view(r % RING, w), in_=_swap_in_ap(src, base, e0, w)
                ).then_inc(load_sem, 16)

        @block.scalar
        def _(scalar):
            for r, (base, e0, w) in enumerate(ring):
                # RAW: ring unit r's load is at position r+1 in the load queue.
                scalar.wait_ge(load_sem, 16 * (r + 2))
                scalar.dma_start(
                    out=_swap_out_ap(dst, base, e0, w), in_=tile_view(r % RING, w)
                ).then_inc(store_sem, 16)
            base, e0, w = pre
            scalar.wait_ge(load_sem, 16)  # prestage load (queue head)
            scalar.dma_start(
                out=_swap_out_ap(dst, base, e0, w), in_=tile_view(RING, w)
            ).then_inc(store_sem, 16)
            scalar.wait_ge(store_sem, 16 * n_units)


def _emit_bounce(nc, src, dst, units):
    """Loads on sync / stores on scalar, NSLOT-deep pipeline over units."""
    n = len(units)
    with (
        nc.sbuf_tensor("tiles", [P, NSLOT * FREE], mybir.dt.float32) as tiles,
        nc.semaphore("load_sem") as load_sem,
        nc.semaphore("store_sem") as store_sem,
        nc.Block() as block,
    ):

        def tile_view(i, w):
            s = (i % NSLOT) * FREE
            return tiles[:, s : s + w]

        @block.sync
        def _(sync):
            for i, (kind, base, e0, w) in enumerate(units):
                if i >= NSLOT:
                    sync.wait_ge(store_sem, 16 * (i - NSLOT + 1))
                if kind == "s":
                    # partition p = (c, j) reads chunk (c, 1-j)'s e-slice
                    in_ap = bass.AP(
                        src,
                        base + TBIT + e0,
                        [[2 * TBIT, PAIRS], [-TBIT, 2], [1, w]],
                    )
                else:
                    in_ap = bass.AP(src, base + e0 * P, [[1, w * P]])
                sync.dma_start(out=tile_view(i, w), in_=in_ap).then_inc(
                    load_sem, 16
                )

        @block.scalar
        def _(scalar):
            for i, (kind, base, e0, w) in enumerate(units):
                scalar.wait_ge(load_sem, 16 * (i + 1))
                if kind == "s":
                    out_ap = bass.AP(dst, base + e0, [[TBIT, P], [1, w]])
                else:
                    out_ap = bass.AP(dst, base + e0 * P, [[1, w * P]])
                scalar.dma_start(
                    out=out_ap, in_=tile_view(i, w)
                ).then_inc(store_sem, 16)
            scalar.wait_ge(store_sem, 16 * n)


def _build_nc(inplace):
    import os as _os

    nc = bass.Bass(target_bir_lowering=False)
    out = nc.dram_tensor("out", (ROWS, DIM), mybir.dt.float32, kind="ExternalOutput")
    if inplace:
        if _os.environ.get("BASS_CNOT_MODE", "mixed") == "bounce":
            _emit_prestage(nc, out, out)
        else:
            _emit_mixed(nc, out)
    else:
        st = nc.dram_tensor(
            "state", (ROWS, DIM), mybir.dt.float32, kind="ExternalInput"
        )
        _emit_bounce(nc, st, out, _units(inplace=False))
    if not nc.is_finalized():
        nc.finalize()
    return nc


def _get_nc(inplace):
    import os as _os

    mode = _os.environ.get("BASS_CNOT_MODE", "mixed") if inplace else "fc"
    key = ("ip" if inplace else "fc", mode)
    if key not in _cache:
        _cache[key] = _build_nc(inplace)
    return _cache[key]


def _run_donated(nc, state):
    """Run `nc` via PJRT shard_map over 8 cores, donating the input state as
    the initial content of the (aliased) output buffer — the same donation
    mechanism run_bass_via_pjrt uses for its zero-filled outputs."""
    import jax
    from jax.experimental.shard_map import shard_map
    from jax.sharding import Mesh, PartitionSpec

    from concourse.bass2jax import (
        _bass_exec_p,
        install_neuronx_cc_hook,
        partition_id_tensor,
    )

    install_neuronx_cc_hook()

    out_names, out_avals = [], []
    for alloc in nc.m.functions[0].allocations:
        if (
            isinstance(alloc, mybir.MemoryLocationSet)
            and alloc.kind == "ExternalOutput"
        ):
            out_names.append(alloc.memorylocations[0].name)
            out_avals.append(
                jax.core.ShapedArray(
                    tuple(alloc.tensor_shape), mybir.dt.np(alloc.dtype)
                )
            )
    partition_name = nc.partition_id_tensor.name if nc.partition_id_tensor else None
    in_names = list(out_names)
    if partition_name is not None:
        in_names.append(partition_name)

    if "donated_fn" not in _cache:

        def _body(buf):
            operands = [buf]
            if partition_name is not None:
                operands.append(partition_id_tensor())
            outs = _bass_exec_p.bind(
                *operands,
                out_avals=tuple(out_avals),
                in_names=tuple(in_names),
                out_names=tuple(out_names),
                lowering_input_output_aliases=(),
                sim_require_finite=True,
                sim_require_nnan=True,
                nc=nc,
            )
            return outs[0]

        devices = jax.devices()[:N_CORES]
        mesh = Mesh(np.asarray(devices), ("core",))
        _cache["donated_fn"] = jax.jit(
            shard_map(
                _body,
                mesh=mesh,
                in_specs=(PartitionSpec("core"),),
                out_specs=PartitionSpec("core"),
                check_rep=False,
            ),
            donate_argnums=(0,),
            keep_unused=True,
        )

    out = _cache["donated_fn"](state)
    return np.asarray(out)


def _sample_ok(state, out, rng, k=2048):
    """Spot-check out[b, j] == state[b, j ^ (1<<13) if bit20(j) else j]."""
    b = rng.integers(0, BATCH, size=k)
    j = rng.integers(0, DIM, size=k)
    src = np.where((j >> C2) & 1 == 1, j ^ TBIT, j)
    return np.array_equal(out[b, j], state[b, src])


def kernel(state, control=3, target=10, num_qubits=24, **_):
    state = np.ascontiguousarray(np.asarray(state, dtype=np.float32))
    assert state.shape == (BATCH, DIM), state.shape
    assert int(control) == 3 and int(target) == 10 and int(num_qubits) == 24

    rng = np.random.default_rng(0)
    try:
        out = _run_donated(_get_nc(inplace=True), state)
        if _sample_ok(state, out, rng):
            return out
    except Exception:
        # Retry once with a fresh jit: a transient dispatch failure before
        # any device execution is cheap to retry; a second failure means the
        # aliasing mechanism is broken here -> full-copy.
        _cache.pop("donated_fn", None)
        try:
            out = _run_donated(_get_nc(inplace=True), state)
            if _sample_ok(state, out, rng):
                return out
        except Exception:
            pass

    # Fallback: full-copy kernel through run_bass_kernel_spmd.
    nc = _get_nc(inplace=False)
    in_maps = [{"state": state[c * ROWS : (c + 1) * ROWS]} for c in range(N_CORES)]
    res = run_bass_kernel_spmd(nc, in_maps, core_ids=list(range(N_CORES)))
    return np.concatenate([r["out"] for r in res.results], axis=0)



# revision 11
# speedup vs baseline: 1.5539x; 1.1742x over previous
"""CNOT permutation kernel for Trainium2 (Bass), 8-core data parallel.

Problem (hardcoded from spec): state (16, 2**24) f32, control=3, target=10,
num_qubits=24.  With c2 = 24-3-1 = 20 and t2 = 24-10-1 = 13:

    out[b, j] = state[b, j ^ (1<<13)]  if (j >> 20) & 1 else state[b, j]

Pure data movement.  Viewing the per-core shard flat (row stride 2**24 is a
multiple of the 2**21 control-bit period, so both rows fuse) as
[blk:16, ctrl:2, c:64, d:2, e:8192]:

    out[blk, 0, c, d, e] = in[blk, 0, c, d, e]      (identity half)
    out[blk, 1, c, d, e] = in[blk, 1, c, 1-d, e]    (swap 8192-elem chunk pairs)

Sharding: batch axis, 2 rows per core (pure data parallel).

Device kernel (in-place "mixed" path): each 4 MiB region is swapped with
HALF the engine traffic of a pure HBM->SBUF->HBM bounce.  The binding
resource is the 16 SDMA engines / SBUF-AXI ports (~27 GB/s each): a
bounced byte transits an engine twice (load + store), a direct DRAM->DRAM
byte only once -- and 32 KiB-descriptor D2D measures at full port line
rate (~26.2 GB/s/engine), NOT the halved rate folklore suggests.  An
in-place 2-cycle swap needs one side as temp, so the optimum is 3 engine
passes per pair: per region, the A-half (64x32 KiB chunks at +0) bounces
through SBUF while the B-half moves B->A-locations as one direct
DRAM->DRAM DMA; the store then writes the SBUF copy to the B-locations.
Engine passes: 64 MiB bounced + 32 MiB D2D = 96 MiB vs 128 MiB all-bounce.

Queues: loads on the Sync HWDGE ring; D2D + stores interleaved (lag 1) on
the Scalar ring; L_r -> D_r -> S_r ordered by completion semaphores (a
DMA's sem fires only when all 16 engines finish, so D_r complete implies
its B-reads are done before S_r overwrites them).  The final region is
prestaged: its load heads the sync queue, its D2D heads the scalar queue,
its store tails it, so the tail drains fully-queued at the 16-engine rate
with no exposed completion receipt (~6 us saved vs a naive tail).

Measured (8-core SPMD, profiled core): ~253 us when the HBM-stack partner
core's window doesn't overlap (engine-bound, zero DMA gaps), ~300-430 us
under partner contention (per-packet HBM stalls; still ahead of the pure
bounce, which models to ~486 us at the same contention).  Roughly 16 us of
that span is fixed NEFF preamble (engine init + all-engine barriers).

1. In-place (default): a single DRAM tensor, pre-initialized with the input
   shard by donating it as the PJRT output buffer (the same donation
   mechanism run_bass_via_pjrt uses for its zero-filled outputs).  Only the
   control-bit=1 half is touched: 64 MiB read + 64 MiB write per core.  The
   identity half is never moved at all.
2. Full-copy (fallback): separate in/out tensors, 128+128 MiB per core.

kernel() runs the in-place path and sample-checks the permutation against
the host input; if the donation aliasing ever fails to hold (output buffer
not seeded with the input), it falls back to the full-copy path.
"""

import numpy as np

import concourse.bass as bass
import concourse.mybir as mybir
from concourse.bass_utils import run_bass_kernel_spmd

NUM_QUBITS = 24
DIM = 1 << NUM_QUBITS
BATCH = 16
N_CORES = 8
ROWS = BATCH // N_CORES  # 2 rows per core
C2 = NUM_QUBITS - 3 - 1  # 20
T2 = NUM_QUBITS - 10 - 1  # 13
CBIT = 1 << C2  # 1048576 elements (4 MiB)
TBIT = 1 << T2  # 8192 elements (32 KiB)
BLK = 2 * CBIT  # control-bit period
NBLK = ROWS * DIM // BLK  # 16 blocks in the fused per-core space

P = 128
PAIRS = CBIT // (2 * TBIT)  # 64 chunk pairs per 4 MiB region

# Units are e-axis slices (each spans all 128 partitions).  BODY_W elems /
# partition per unit in steady state; the first region ramps up so the
# store stream starts early.
BODY_W = 8192  # 4 MiB units: body stores collapse to one contiguous run
NSLOT = 6  # SBUF slots: 6 x 32 KiB/partition = 192 KiB of ~208 usable
RING = NSLOT - 1  # 5-deep ring; slot RING is the prestage slot
FREE = BODY_W
MIX_RING = 11  # mixed path: 11 x 16 KiB ring + 16 KiB prestage = 192 KiB
# Head ramp: w=4096 keeps descriptors at 16 KiB (w<2048 makes 2-8 KiB
# descriptors that crawl under neighbor-core contention).
HEAD_SPLIT = [4096, 4096]
TAIL_SPLIT = [4096, 4096]  # fallback (full-copy) path only

_cache = {}


def _region_units(kind, base, splits):
    out, e0 = [], 0
    for w in splits:
        out.append((kind, base, e0, w))
        e0 += w
    assert e0 == TBIT
    return out


def _units(inplace):
    """Yield (kind, region_base, e0, w): the unit covers the e in [e0,e0+w)
    slice of all 128 TBIT-chunks of its 4 MiB region.  kind 's'=swap,
    'i'=identity."""
    regions = []
    for b in range(NBLK):
        if not inplace:
            regions.append(("i", b * BLK))
        regions.append(("s", b * BLK + CBIT))
    if not inplace:
        # Ramp regions get strided stores when split mid-chunk -- except
        # identity regions, whose stores stay contiguous at any width.  Put
        # an identity region last so both ramps are identity-only and every
        # swap store is one contiguous 4 MiB run.
        regions.append(regions.pop(-2))
    body = [BODY_W] * (TBIT // BODY_W)
    units = []
    for r, (kind, base) in enumerate(regions):
        if r == 0:
            splits = HEAD_SPLIT
        elif r == len(regions) - 1:
            splits = TAIL_SPLIT
        else:
            splits = body
        units.extend(_region_units(kind, base, splits))
    return units


def _swap_in_ap(src, base, e0, w):
    # partition p = (c, j) reads chunk (c, 1-j)'s e-slice
    return bass.AP(src, base + TBIT + e0, [[2 * TBIT, PAIRS], [-TBIT, 2], [1, w]])


def _swap_out_ap(dst, base, e0, w):
    return bass.AP(dst, base + e0, [[TBIT, P], [1, w]])


# Mixed-path APs: the A-half of a region (chunks at +0 in each 2*TBIT block)
# bounces through SBUF as a [128, HALF_W] tile (each 32 KiB chunk split
# across 2 partitions so all 16 engine ports engage); the B-half moves
# HBM->HBM directly.
HALF_W = TBIT // 2  # 4096


def _half_dram_ap(t, base):
    # partition p = (c, h): run at base + c*2*TBIT + h*HALF_W, 16 KiB each
    return bass.AP(t, base, [[2 * TBIT, PAIRS], [HALF_W, 2], [1, HALF_W]])


def _d2d_ap(t, base):
    # 64 contiguous 32 KiB runs at stride 2*TBIT.  BASS_CNOT_D2D16=1 splits
    # each run into 2 descriptors of 16 KiB (shorter engine-blocking per
    # packet when HBM is congested).
    import os as _os

    if _os.environ.get("BASS_CNOT_D2D16", "0") == "1":
        return bass.AP(t, base, [[2 * TBIT, PAIRS], [HALF_W, 2], [1, HALF_W]])
    return bass.AP(t, base, [[2 * TBIT, PAIRS], [1, TBIT]])


def _emit_mixed(nc, buf):
    """In-place swap, half bounced / half direct DRAM->DRAM.

    Per 4 MiB region r (64 pairs of 32 KiB chunks A@+0 / B@+TBIT):
      L_r: A-half (2 MiB) -> SBUF ring slot        [sync queue]
      D_r: B-half -> A-locations, direct HBM->HBM  [scalar queue]
      S_r: SBUF (A data) -> B-locations            [scalar queue]
    with L_r -> D_r -> S_r enforced by completion semaphores.  D2D moves
    payload at ~20.8 GB/s/engine (measured) vs 27.2 streaming, but only
    transits the engine once, so total engine-time is ~0.82x of the pure
    bounce.  The final region is prestaged: its load is first in the sync
    queue, its D2D second in the scalar queue, and its store last, so the
    tail is fully-queued stores at the 16-engine rate.
    """
    n_ring = NBLK - 1  # regions 0..14 through the ring
    pre_base = (NBLK - 1) * BLK + CBIT
    ring_bases = [b * BLK + CBIT for b in range(n_ring)]

    with (
        nc.sbuf_tensor("ring", [P, MIX_RING * HALF_W], mybir.dt.float32) as ring,
        nc.sbuf_tensor("pre", [P, HALF_W], mybir.dt.float32) as pre,
        nc.semaphore("load_sem") as load_sem,
        nc.semaphore("d2d_sem") as d2d_sem,
        nc.semaphore("store_sem") as store_sem,
        nc.Block() as block,
    ):

        def slot(r):
            s = (r % MIX_RING) * HALF_W
            return ring[:, s : s + HALF_W]

        @block.sync
        def _(sync):
            # Prestage load: final region's A-half into the dedicated tile.
            sync.dma_start(
                out=pre[:, :], in_=_half_dram_ap(buf, pre_base)
            ).then_inc(load_sem, 16)
            for r, base in enumerate(ring_bases):
                if r >= MIX_RING:
                    # WAR: slot last used by ring unit r-MIX_RING, whose
                    # store is at position r-MIX_RING in store completion.
                    sync.wait_ge(store_sem, 16 * (r - MIX_RING + 1))
                sync.dma_start(
                    out=slot(r), in_=_half_dram_ap(buf, base)
                ).then_inc(load_sem, 16)

        @block.scalar
        def _(scalar):
            # Prestage D2D: B-half -> A-locations of the final region.
            scalar.wait_ge(load_sem, 16)  # prestage load (sync queue head)
            scalar.dma_start(
                out=_d2d_ap(buf, pre_base), in_=_d2d_ap(buf, pre_base + TBIT)
            ).then_inc(d2d_sem, 16)
            # Interleave D_r / S_{r-1} with lag 1 so receipt waits are
            # resolved by the time the sequencer reaches them.
            for r, base in enumerate(ring_bases):
                # D_r needs L_r complete (position r+1 in the load queue).
                scalar.wait_ge(load_sem, 16 * (r + 2))
                scalar.dma_start(
                    out=_d2d_ap(buf, base), in_=_d2d_ap(buf, base + TBIT)
                ).then_inc(d2d_sem, 16)
                if r >= 1:
                    pb = ring_bases[r - 1]
                    # S_{r-1} needs D_{r-1} complete (position r in d2d).
                    scalar.wait_ge(d2d_sem, 16 * (r + 1))
                    scalar.dma_start(
                        out=_half_dram_ap(buf, pb + TBIT), in_=slot(r - 1)
                    ).then_inc(store_sem, 16)
            # Last ring store.
            scalar.wait_ge(d2d_sem, 16 * (n_ring + 1))
            scalar.dma_start(
                out=_half_dram_ap(buf, ring_bases[-1] + TBIT), in_=slot(n_ring - 1)
            ).then_inc(store_sem, 16)
            # Prestage store: A data -> B-locations of the final region.
            scalar.wait_ge(d2d_sem, 16)  # prestage D2D (scalar queue head)
            scalar.dma_start(
                out=_half_dram_ap(buf, pre_base + TBIT), in_=pre[:, :]
            ).then_inc(store_sem, 16)
            scalar.wait_ge(store_sem, 16 * NBLK)


def _emit_prestage(nc, src, dst):
    """In-place swap pipeline with the final region prestaged.

    Ring units (region 0 split per HEAD_SPLIT, regions 1..NBLK-2 whole)
    cycle through RING SBUF slots; the final region is loaded FIRST into
    slot RING and stored LAST, so the tail is a fully-queued 4 MiB store
    with no exposed load-completion receipt."""
    ring = []  # (base, e0, w)
    base0 = 0 * BLK + CBIT
    e0 = 0
    for w in HEAD_SPLIT:
        ring.append((base0, e0, w))
        e0 += w
    assert e0 == TBIT
    for b in range(1, NBLK - 1):
        ring.append((b * BLK + CBIT, 0, BODY_W))
    pre = ((NBLK - 1) * BLK + CBIT, 0, BODY_W)  # prestaged final region
    n_units = len(ring) + 1

    with (
        nc.sbuf_tensor("tiles", [P, NSLOT * FREE], mybir.dt.float32) as tiles,
        nc.semaphore("load_sem") as load_sem,
        nc.semaphore("store_sem") as store_sem,
        nc.Block() as block,
    ):

        def tile_view(slot, w):
            s = slot * FREE
            return tiles[:, s : s + w]

        @block.sync
        def _(sync):
            # Prestage load: final region into the dedicated slot.
            base, e0, w = pre
            sync.dma_start(
                out=tile_view(RING, w), in_=_swap_in_ap(src, base, e0, w)
            ).then_inc(load_sem, 16)
            for r, (base, e0, w) in enumerate(ring):
                if r >= RING:
                    # WAR: slot r%RING was last used by ring unit r-RING,
                    # whose store is at position r-RING in the store queue.
                    sync.wait_ge(store_sem, 16 * (r - RING + 1))
                sync.dma_start(
                    out=tile_view(r % RING, w), in_=_swap_in_ap(src, base, e0, w)
                ).then_inc(load_sem, 16)

        @block.scalar
        def _(scalar):
            for r, (base, e0, w) in enumerate(ring):
                # RAW: ring unit r's load is at position r+1 in the load queue.
                scalar.wait_ge(load_sem, 16 * (r + 2))
                scalar.dma_start(
                    out=_swap_out_ap(dst, base, e0, w), in_=tile_view(r % RING, w)
                ).then_inc(store_sem, 16)
            base, e0, w = pre
            scalar.wait_ge(load_sem, 16)  # prestage load (queue head)
            scalar.dma_start(
                out=_swap_out_ap(dst, base, e0, w), in_=tile_view(RING, w)
            ).then_inc(store_sem, 16)
            scalar.wait_ge(store_sem, 16 * n_units)


def _emit_bounce(nc, src, dst, units):
    """Loads on sync / stores on scalar, NSLOT-deep pipeline over units."""
    n = len(units)
    with (
        nc.sbuf_tensor("tiles", [P, NSLOT * FREE], mybir.dt.float32) as tiles,
        nc.semaphore("load_sem") as load_sem,
        nc.semaphore("store_sem") as store_sem,
        nc.Block() as block,
    ):

        def tile_view(i, w):
            s = (i % NSLOT) * FREE
            return tiles[:, s : s + w]

        @block.sync
        def _(sync):
            for i, (kind, base, e0, w) in enumerate(units):
                if i >= NSLOT:
                    sync.wait_ge(store_sem, 16 * (i - NSLOT + 1))
                if kind == "s":
                    # partition p = (c, j) reads chunk (c, 1-j)'s e-slice
                    in_ap = bass.AP(
                        src,
                        base + TBIT + e0,
                        [[2 * TBIT, PAIRS], [-TBIT, 2], [1, w]],
                    )
                else:
                    in_ap = bass.AP(src, base + e0 * P, [[1, w * P]])
                sync.dma_start(out=tile_view(i, w), in_=in_ap).then_inc(
                    load_sem, 16
                )

        @block.scalar
        def _(scalar):
            for i, (kind, base, e0, w) in enumerate(units):
                scalar.wait_ge(load_sem, 16 * (i + 1))
                if kind == "s":
                    out_ap = bass.AP(dst, base + e0, [[TBIT, P], [1, w]])
                else:
                    out_ap = bass.AP(dst, base + e0 * P, [[1, w * P]])
                scalar.dma_start(
                    out=out_ap, in_=tile_view(i, w)
                ).then_inc(store_sem, 16)
            scalar.wait_ge(store_sem, 16 * n)


def _build_nc(inplace):
    import os as _os

    nc = bass.Bass(target_bir_lowering=False)
    out = nc.dram_tensor("out", (ROWS, DIM), mybir.dt.float32, kind="ExternalOutput")
    if inplace:
        if _os.environ.get("BASS_CNOT_MODE", "mixed") == "bounce":
            _emit_prestage(nc, out, out)
        else:
            _emit_mixed(nc, out)
    else:
        st = nc.dram_tensor(
            "state", (ROWS, DIM), mybir.dt.float32, kind="ExternalInput"
        )
        _emit_bounce(nc, st, out, _units(inplace=False))
    if not nc.is_finalized():
        nc.finalize()
    return nc


def _get_nc(inplace):
    import os as _os

    mode = _os.environ.get("BASS_CNOT_MODE", "mixed") if inplace else "fc"
    key = ("ip" if inplace else "fc", mode)
    if key not in _cache:
        _cache[key] = _build_nc(inplace)
    return _cache[key]


def _run_donated(nc, state):
    """Run `nc` via PJRT shard_map over 8 cores, donating the input state as
    the initial content of the (aliased) output buffer — the same donation
    mechanism run_bass_via_pjrt uses for its zero-filled outputs."""
    import jax
    from jax.experimental.shard_map import shard_map
    from jax.sharding import Mesh, PartitionSpec

    from concourse.bass2jax import (
        _bass_exec_p,
        install_neuronx_cc_hook,
        partition_id_tensor,
    )

    install_neuronx_cc_hook()

    out_names, out_avals = [], []
    for alloc in nc.m.functions[0].allocations:
        if (
            isinstance(alloc, mybir.MemoryLocationSet)
            and alloc.kind == "ExternalOutput"
        ):
            out_names.append(alloc.memorylocations[0].name)
            out_avals.append(
                jax.core.ShapedArray(
                    tuple(alloc.tensor_shape), mybir.dt.np(alloc.dtype)
                )
            )
    partition_name = nc.partition_id_tensor.name if nc.partition_id_tensor else None
    in_names = list(out_names)
    if partition_name is not None:
        in_names.append(partition_name)

    if "donated_fn" not in _cache:

        def _body(buf):
            operands = [buf]
            if partition_name is not None:
                operands.append(partition_id_tensor())
            outs = _bass_exec_p.bind(
                *operands,
                out_avals=tuple(out_avals),
                in_names=tuple(in_names),
                out_names=tuple(out_names),
                lowering_input_output_aliases=(),
                sim_require_finite=True,
                sim_require_nnan=True,
                nc=nc,
            )
            return outs[0]

        devices = jax.devices()[:N_CORES]
        mesh = Mesh(np.asarray(devices), ("core",))
        _cache["donated_fn"] = jax.jit(
            shard_map(
                _body,
                mesh=mesh,
                in_specs=(PartitionSpec("core"),),
                out_specs=PartitionSpec("core"),
                check_rep=False,
            ),
            donate_argnums=(0,),
            keep_unused=True,
        )

    out = _cache["donated_fn"](state)
    return np.asarray(out)


def _sample_ok(state, out, rng, k=2048):
    """Spot-check out[b, j] == state[b, j ^ (1<<13) if bit20(j) else j]."""
    b = rng.integers(0, BATCH, size=k)
    j = rng.integers(0, DIM, size=k)
    src = np.where((j >> C2) & 1 == 1, j ^ TBIT, j)
    return np.array_equal(out[b, j], state[b, src])


def kernel(state, control=3, target=10, num_qubits=24, **_):
    state = np.ascontiguousarray(np.asarray(state, dtype=np.float32))
    assert state.shape == (BATCH, DIM), state.shape
    assert int(control) == 3 and int(target) == 10 and int(num_qubits) == 24

    rng = np.random.default_rng(0)
    try:
        out = _run_donated(_get_nc(inplace=True), state)
        if _sample_ok(state, out, rng):
            return out
    except Exception:
        # Retry once with a fresh jit: a transient dispatch failure before
        # any device execution is cheap to retry; a second failure means the
        # aliasing mechanism is broken here -> full-copy.
        _cache.pop("donated_fn", None)
        try:
            out = _run_donated(_get_nc(inplace=True), state)
            if _sample_ok(state, out, rng):
                return out
        except Exception:
            pass

    # Fallback: full-copy kernel through run_bass_kernel_spmd.
    nc = _get_nc(inplace=False)
    in_maps = [{"state": state[c * ROWS : (c + 1) * ROWS]} for c in range(N_CORES)]
    res = run_bass_kernel_spmd(nc, in_maps, core_ids=list(range(N_CORES)))
    return np.concatenate([r["out"] for r in res.results], axis=0)

